# revision 16
# baseline (speedup 1.0000x reference)
"""Trainium2 Bass kernel for nn_CrossModalAttention.

Problem: bidirectional cross-attention between two (B, C, H, W) feature maps.
  B=4, C=256, H=W=64 -> N=4096 pixels, HID=64.
  For each direction:  q = Wq@xq, k = Wk@xkv, v = Wv@xkv (1x1 convs),
  attn = softmax_m(q^T k), out = xq + gamma * (v @ attn^T).

Sharding: 2 directions x 4 batches = 8 independent units, one per NeuronCore.

Per-core layout: compute S^T tiles [m(part)=128, n(free)=512] via
matmul(lhsT=k_tile, rhs=q_tile) (contraction over HID=64 on partitions), exp on
ScalarE (logits are bounded ~ +-56, so exp in f32 needs no max-subtraction),
then accumulate Y[c, n] = sum_m x2T[m, c]^T expS^T[m, n] in PSUM across the m
blocks; U = Wv @ Y afterwards (v-projection folded past the attention sum by
associativity). Final: out = xq + (gamma/d)*U + gamma*bv.

All matmul operands are bf16 (1 cycle/row at 2.4 GHz; f32r is the same rate but
pays a slower 4-byte weight load). S matmuls are emitted one m-pair ahead of
the PV matmuls so the PE never waits on the ScalarE exp. S tiles are paired
[128, 2, 512] across two PSUM banks so one ACT instruction exps 1024 elems.
"""

import sys

if "/opt/trn_rl_repo" not in sys.path:
    sys.path.insert(0, "/opt/trn_rl_repo")

import ml_dtypes
import numpy as np

B = 4
C = 256
HID = 64
N = 4096          # H*W
P = 128           # SBUF partitions
NT = 512          # n-tile (matmul moving free dim)
N_NT = N // NT    # 8
MB = 128          # m-block (PV contraction tile)
N_MB = N // MB    # 32
NPAIR = N_MB // 2  # 16 m-pairs per n-tile
CA = C // P       # 2 c-chunks / c-blocks

_CACHE = {}


def _build_program():
    import concourse.bass as bass
    import concourse.mybir as mybir
    from concourse import tile

    f32 = mybir.dt.float32
    f32r = mybir.dt.float32r
    bf16 = mybir.dt.bfloat16
    AF = mybir.ActivationFunctionType

    nc = bass.Bass("TRN2", target_bir_lowering=False, debug=False)

    xq_d = nc.dram_tensor("xq", (C, N), bf16, kind="ExternalInput")
    xkv_d = nc.dram_tensor("xkv", (C, N), bf16, kind="ExternalInput")
    wqT_d = nc.dram_tensor("wqT", (C, HID), bf16, kind="ExternalInput")
    wkT_d = nc.dram_tensor("wkT", (C, HID), bf16, kind="ExternalInput")
    wvT_d = nc.dram_tensor("wvT", (C, C), bf16, kind="ExternalInput")
    bq_d = nc.dram_tensor("bq", (P, 1), f32, kind="ExternalInput")   # bq tiled x2
    bk_d = nc.dram_tensor("bk", (P, 1), f32, kind="ExternalInput")   # bk tiled x2
    gbv_d = nc.dram_tensor("gbv", (C, 1), f32, kind="ExternalInput")      # gamma * bv
    rgam_d = nc.dram_tensor("rgam", (1, 1), f32, kind="ExternalInput")    # 1 / gamma
    x2tb_d = nc.dram_tensor("x2tb", (N, C), bf16, kind="ExternalInput")   # bf16 xkv^T
    out_d = nc.dram_tensor("out", (C, N), f32, kind="ExternalOutput")

    # c = a*128 + p views
    xq_r = xq_d[:].rearrange("(a p) n -> p a n", p=P)
    xkv_r = xkv_d[:].rearrange("(a p) n -> p a n", p=P)
    wqT_r = wqT_d[:].rearrange("(a p) h -> p a h", p=P)
    wkT_r = wkT_d[:].rearrange("(a p) h -> p a h", p=P)
    wvT_r = wvT_d[:].rearrange("(a p) c -> p a c", p=P)
    gbv_r = gbv_d[:].rearrange("(a p) one -> p (a one)", p=P)
    out_r = out_d[:].rearrange("(a p) n -> p a n", p=P)
    x2t_r = x2tb_d[:].rearrange("(mb p) c -> p mb c", p=P)

    with tile.TileContext(nc) as tc:
        with (
            tc.tile_pool(name="const", bufs=1) as const,
            tc.tile_pool(name="xin", bufs=1) as xin,
            tc.tile_pool(name="qk", bufs=1) as qk,
            tc.tile_pool(name="vtp", bufs=1) as vtp,
            tc.tile_pool(name="work", bufs=3) as work,
            tc.tile_pool(name="ep", bufs=2) as ep,
            tc.tile_pool(name="dram", bufs=2, space="DRAM") as dram,
            tc.tile_pool(name="psum", bufs=1, space="PSUM") as psum,
        ):
            # ---- constants / weights (ACT HWDGE queue; wk first for k-proj) --
            wk_sb = const.tile([P, CA, HID], bf16, tag="wk")
            nc.scalar.dma_start(wk_sb[:], wkT_r)
            wq_sb = const.tile([P, CA, HID], bf16, tag="wq")
            nc.scalar.dma_start(wq_sb[:], wqT_r)
            wv_sb = const.tile([P, CA, C], bf16, tag="wv")
            nc.scalar.dma_start(wv_sb[:], wvT_r)
            bq_sb = const.tile([P, 1], f32, tag="bq")
            nc.scalar.dma_start(bq_sb[:], bq_d[:])
            bk_sb = const.tile([P, 1], f32, tag="bk")
            nc.scalar.dma_start(bk_sb[:], bk_d[:])
            gbv_sb = const.tile([P, CA], f32, tag="gbv")
            nc.scalar.dma_start(gbv_sb[:], gbv_r)
            rgam_sb = const.tile([1, 1], f32, tag="rgam")
            nc.scalar.dma_start(rgam_sb[:], rgam_d[:])
            ones_sb = const.tile([P, 1], bf16, tag="ones")
            nc.vector.memset(ones_sb[:], 1.0)
            onesb_sb = const.tile([1, P], bf16, tag="onesb")
            nc.vector.memset(onesb_sb[:], 1.0)

            # ---- x loads (chunked for DMA/compute overlap) ----
            # xq chunk 0 first: it gates the very first q-projection.
            xq_sb = xin.tile([P, CA, N], bf16, tag="xq")
            xkv_sb = xin.tile([P, CA, N], bf16, tag="xkv")
            NCH = 1024
            for a in range(CA):
                nc.sync.dma_start(xq_sb[:, a, 0:NCH], xq_r[:, a, 0:NCH])
            for h in range(N // NCH):
                sl = slice(h * NCH, (h + 1) * NCH)
                for a in range(CA):
                    nc.sync.dma_start(xkv_sb[:, a, sl], xkv_r[:, a, sl])
            # X2^T tiles [m, c_in] (bf16) for the Y = X2 @ E matmuls
            # (transpose+cast on the host); SWDGE queue, parallel with sync.
            x2t_sb = vtp.tile([P, N_MB, C], bf16, tag="x2t")
            for mb in range(0, N_MB, 4):
                nc.gpsimd.dma_start(
                    x2t_sb[:, mb : mb + 4, :], x2t_r[:, mb : mb + 4, :]
                )
            for h in range(1, N // NCH):
                sl = slice(h * NCH, (h + 1) * NCH)
                for a in range(CA):
                    nc.sync.dma_start(xq_sb[:, a, sl], xq_r[:, a, sl])

            q_sb = qk.tile([HID, N], bf16, tag="q")
            k_sb = qk.tile([HID, N], bf16, tag="k")

            def _kproj(j):
                # k-projection for n-tiles (2j, 2j+1), packed in one PSUM bank:
                # rows 0:64 <- nt 2j, rows 64:128 <- nt 2j+1
                kp = psum.tile([P, 2, NT], f32, tag="stp", bufs=2, name=f"kp_{j}")
                for half, ntj in ((0, 2 * j), (1, 2 * j + 1)):
                    ntsl = slice(ntj * NT, (ntj + 1) * NT)
                    rows = slice(half * HID, half * HID + HID)
                    for a in range(CA):
                        nc.tensor.matmul(
                            kp[rows, 0, :],
                            lhsT=wk_sb[:, a, :],
                            rhs=xkv_sb[:, a, ntsl],
                            start=(a == 0),
                            stop=(a == CA - 1),
                        )
                    nc.vector.tensor_scalar_add(
                        k_sb[:, ntsl], kp[rows, 0, :], bk_sb[rows]
                    )

            def _qproj(i):
                # q-projection for n-tiles (2i, 2i+1) packed into one bank
                qp = psum.tile([P, NT], f32, tag="mp", bufs=1, name=f"qp_{i}")
                for half, nti in ((0, 2 * i), (1, 2 * i + 1)):
                    ntsl = slice(nti * NT, (nti + 1) * NT)
                    rows = slice(half * HID, half * HID + HID)
                    for a in range(CA):
                        nc.tensor.matmul(
                            qp[rows, :],
                            lhsT=wq_sb[:, a, :],
                            rhs=xq_sb[:, a, ntsl],
                            start=(a == 0),
                            stop=(a == CA - 1),
                        )
                    nc.vector.tensor_scalar_add(
                        q_sb[:, ntsl], qp[rows, :], bq_sb[rows]
                    )

            _kproj(0)
            _qproj(0)

            # ---- attention ----
            GP = 4            # m-pairs per denominator group (8 m-blocks)
            N_G = NPAIR // GP  # 4 groups per n-tile

            def _epi_a(nt, y0, y1, dp):
                # After nt's loop: free the y banks (DVE copies), and move
                # d/gamma off PSUM (ACT copy; f32r so the broadcast matmul can
                # consume it at full rate).
                yb0 = ep.tile([P, NT], bf16, tag="yb0", name=f"yb0_{nt}")
                nc.vector.tensor_scalar_add(yb0[:], y0[:], 0.0)
                yb1 = ep.tile([P, NT], bf16, tag="yb1", name=f"yb1_{nt}")
                nc.vector.tensor_scalar_add(yb1[:], y1[:], 0.0)
                rd = ep.tile([1, NT], f32, tag="rd", name=f"rd_{nt}")
                nc.scalar.activation(rd[:], dp[:], AF.Copy, scale=rgam_sb[:])
                rr = ep.tile([1, NT], f32, tag="rr", name=f"rr_{nt}")
                nc.vector.reciprocal(rr[:], rd[:])
                rb = ep.tile([1, NT], bf16, tag="rb", name=f"rb_{nt}")
                nc.vector.tensor_scalar_add(rb[:], rr[:], 0.0)
                return yb0, yb1, rb

            def _epi_bcast(nt, state):
                # Partition-broadcast gamma/d (1KB bf16) via a DRAM roundtrip.
                yb0, yb1, rb = state
                dscr = dram.tile([1, NT], bf16, tag="dscr", name=f"dscr_{nt}")
                nc.sync.dma_start(dscr[:], rb[:])
                rdb = ep.tile([P, NT], bf16, tag="rdb", name=f"rdb_{nt}")
                nc.sync.dma_start(rdb[:], dscr[:].broadcast_to((P, NT)))
                return yb0, yb1, rdb

            def _epi_b(nt, cb, state):
                # U[cb] = Wv[cb] @ Y  (2 accumulating matmuls), then
                # out[c, n] = xq + rdb[n] * U[c, n] + gamma*bv[c]
                yb0, yb1, rdb = state
                ntsl = slice(nt * NT, (nt + 1) * NT)
                ups = psum.tile([P, NT], f32, tag="mp", bufs=1, name=f"ups_{nt}_{cb}")
                nc.tensor.matmul(
                    ups[:], lhsT=wv_sb[:, 0, cb * P : (cb + 1) * P],
                    rhs=yb0[:], start=True, stop=False,
                )
                nc.tensor.matmul(
                    ups[:], lhsT=wv_sb[:, 1, cb * P : (cb + 1) * P],
                    rhs=yb1[:], start=False, stop=True,
                )
                t = ep.tile([P, NT], f32, tag="t", name=f"t_{nt}_{cb}")
                nc.vector.tensor_mul(t[:], ups[:], rdb[:])
                o = ep.tile([P, NT], f32, tag="o", name=f"o_{nt}_{cb}")
                nc.vector.scalar_tensor_tensor(
                    o[:],
                    in0=t[:],
                    scalar=gbv_sb[:, cb : cb + 1],
                    in1=xq_sb[:, cb, ntsl],
                    op0=mybir.AluOpType.add,
                    op1=mybir.AluOpType.add,
                )
                nc.sync.dma_start(out_r[:, cb, ntsl], o[:])

            prev = [None]   # (nt, state) of the previous n-tile

            for nt in range(N_NT):
                ntsl = slice(nt * NT, (nt + 1) * NT)
                y0 = psum.tile([P, NT], f32, tag="y", bufs=2, name=f"y0_{nt}")
                y1 = psum.tile([P, NT], f32, tag="y", bufs=2, name=f"y1_{nt}")
                ddt = psum.tile([P, NT], f32, tag="dd", bufs=1, name=f"dp_{nt}")
                dp = ddt[0:1, :]

                stp = {}

                def _smm(p, nt=nt, ntsl=ntsl, stp=stp):
                    # S^T pair p: two matmuls [m=128, n=512] into one 2-bank
                    # PSUM tile; contraction over HID=64 partitions.
                    s = psum.tile([P, 2, NT], f32, tag="stp", bufs=2,
                                  name=f"stp_{nt}_{p}")
                    for j in range(2):
                        mb = 2 * p + j
                        msl = slice(mb * MB, (mb + 1) * MB)
                        nc.tensor.matmul(
                            s[:, j, :],
                            lhsT=k_sb[:, msl],
                            rhs=q_sb[:, ntsl],
                            start=True,
                            stop=True,
                        )
                    stp[p] = s

                _smm(0)
                _smm(1)

                acc = None
                n_dmm = 0
                for p in range(NPAIR):
                    # exp of pair p (one ACT op over both PSUM banks)
                    ex = work.tile([P, 2, NT], bf16, tag="expst",
                                   name=f"ex_{nt}_{p}")
                    nc.scalar.activation(ex[:], stp.pop(p)[:], AF.Exp)

                    # hooks: k-proj (nt 0), q-proj, deferred epilogue of nt-1
                    if nt == 0 and p in (0, 4, 8) and (p // 4 + 1) < 4:
                        _kproj(p // 4 + 1)
                    if p == 1 and prev[0] is not None:
                        prev[0] = (prev[0][0], _epi_bcast(prev[0][0], prev[0][1]))
                    if nt % 2 == 1 and p == 1 and nt < N_NT - 1:
                        _qproj((nt + 1) // 2)
                    if p == 4 and prev[0] is not None:
                        _epi_b(prev[0][0], 0, prev[0][1])
                    if p == 8 and prev[0] is not None:
                        _epi_b(prev[0][0], 1, prev[0][1])
                        prev[0] = None

                    # next S pair (keeps the PE fed while ACT exps pair p)
                    if p + 2 < NPAIR:
                        _smm(p + 2)

                    # PV: y[c] += x2t[mb]^T @ exp(S^T[mb]) for both halves
                    for j in range(2):
                        mb = 2 * p + j
                        first, last = (mb == 0), (mb == N_MB - 1)
                        nc.tensor.matmul(
                            y0[:], lhsT=x2t_sb[:, mb, 0:P], rhs=ex[:, j, :],
                            start=first, stop=last,
                        )
                        nc.tensor.matmul(
                            y1[:], lhsT=x2t_sb[:, mb, P:C], rhs=ex[:, j, :],
                            start=first, stop=last,
                        )

                    # denominator: bf16 running sum over the group's pairs,
                    # then 2 ones-matmuls per group accumulated into dp
                    if p % GP == 0:
                        acc = ex
                    else:
                        s_ = work.tile([P, 2, NT], bf16, tag=f"dacc{p % 2}",
                                       bufs=2, name=f"ds_{nt}_{p}")
                        nc.vector.tensor_add(s_[:], acc[:], ex[:])
                        acc = s_
                    if (p + 1) % GP == 0:
                        for j in range(2):
                            n_dmm += 1
                            nc.tensor.matmul(
                                dp[:], lhsT=ones_sb[:], rhs=acc[:, j, :],
                                start=(n_dmm == 1), stop=(n_dmm == 2 * N_G),
                            )
                        acc = None

                state = _epi_a(nt, y0, y1, dp)
                prev[0] = (nt, state)

            state = _epi_bcast(prev[0][0], prev[0][1])
            _epi_b(prev[0][0], 0, state)
            _epi_b(prev[0][0], 1, state)

    return nc


def _split_excess_waits(nc):
    """The pinned walrus build only encodes 1 sync-wait per instruction;
    newer concourse attaches more. Hoist excess waits onto same-engine NoOps
    inserted immediately before the over-limit instruction (semantically
    identical: same engine, same program position)."""
    import concourse.mybir as mybir
    import bass_rust

    ctr = 0
    for bbl in nc.m.functions[0].blocks:
        il = bbl.instructions
        i = 0
        while i < len(il):
            inst = il[i]
            si = inst.sync_info
            limit = 1
            if si is not None and len(si.on_wait) > limit:
                waits = list(si.on_wait)
                extra = waits[limit:]
                for j in range(0, len(extra), 1):
                    nop = mybir.InstNoOp(name=f"I-wsplit-{ctr}", ins=[], outs=[])
                    ctr += 1
                    nop.engine = inst.engine
                    nop.sync_info = bass_rust.SyncInfo(
                        on_wait=[extra[j]], on_update=[]
                    )
                    il.insert(i, nop)
                    i += 1
                si.on_wait = waits[:limit]
                inst.sync_info = si
            i += 1
    return ctr


def _get_program():
    if "nc" not in _CACHE:
        _CACHE["nc"] = _build_program()
    return _CACHE["nc"]


def _get_program_hw():
    """Program with the walrus sync-wait workaround applied (breaks CoreSim's
    race detector, so only applied for hardware runs)."""
    nc = _get_program()
    if not _CACHE.get("split_done"):
        _split_excess_waits(nc)
        _CACHE["split_done"] = True
    return nc


def _make_in_maps(x1, x2, Wq, bq, Wk, bk, Wv, bv, gamma):
    g = float(np.asarray(gamma).reshape(-1)[0])
    bf = ml_dtypes.bfloat16
    shared = {
        "wqT": np.ascontiguousarray(Wq.T).astype(bf),
        "wkT": np.ascontiguousarray(Wk.T).astype(bf),
        "wvT": np.ascontiguousarray(Wv.T).astype(bf),
        "bq": np.tile(np.asarray(bq, dtype=np.float32).reshape(HID, 1), (2, 1)),
        "bk": np.tile(np.asarray(bk, dtype=np.float32).reshape(HID, 1), (2, 1)),
        "gbv": (g * np.asarray(bv, dtype=np.float32)).reshape(C, 1),
        "rgam": np.array([[1.0 / g if g != 0.0 else 0.0]], dtype=np.float32),
    }
    in_maps = []
    for d in range(2):
        src_q, src_kv = (x1, x2) if d == 0 else (x2, x1)
        for b in range(B):
            xkv_f32 = np.ascontiguousarray(src_kv[b].reshape(C, N), dtype=np.float32)
            in_maps.append(
                {
                    "xq": np.ascontiguousarray(
                        src_q[b].reshape(C, N), dtype=np.float32
                    ).astype(bf),
                    "xkv": xkv_f32.astype(bf),
                    "x2tb": np.ascontiguousarray(xkv_f32.T).astype(bf),
                    **shared,
                }
            )
    return in_maps


def kernel(x1, x2, Wq, bq, Wk, bk, Wv, bv, gamma, _want_results=False):
    x1 = np.asarray(x1, dtype=np.float32)
    x2 = np.asarray(x2, dtype=np.float32)
    nc = _get_program_hw()
    in_maps = _make_in_maps(x1, x2, Wq, bq, Wk, bk, Wv, bv, gamma)

    from concourse.bass_utils import run_bass_kernel_spmd

    res = run_bass_kernel_spmd(nc, in_maps, core_ids=list(range(2 * B)))
    outs = [r["out"].reshape(C, 64, 64) for r in res.results]
    out1 = np.stack(outs[:B]).astype(np.float32)
    out2 = np.stack(outs[B:]).astype(np.float32)
    if _want_results:
        return (out1, out2), res
    return (out1, out2)


# revision 21
# speedup vs baseline: 1.0380x; 1.0380x over previous
"""Trainium2 Bass kernel for nn_CrossModalAttention.

Problem: bidirectional cross-attention between two (B, C, H, W) feature maps.
  B=4, C=256, H=W=64 -> N=4096 pixels, HID=64.
  For each direction:  q = Wq@xq, k = Wk@xkv, v = Wv@xkv (1x1 convs),
  attn = softmax_m(q^T k), out = xq + gamma * (v @ attn^T).

Sharding: 2 directions x 4 batches = 8 independent units, one per NeuronCore.

Per-core layout: compute S^T tiles [m(part)=128, n(free)=512] via
matmul(lhsT=k_tile, rhs=q_tile) (contraction over HID=64 on partitions), exp on
ScalarE (logits are bounded ~ +-56, so exp in f32 needs no max-subtraction),
then accumulate Y[c, n] = sum_m x2T[m, c]^T expS^T[m, n] in PSUM across the m
blocks; U = Wv @ Y afterwards (v-projection folded past the attention sum by
associativity). Final: out = xq + (gamma/d)*U + gamma*bv.

All matmul operands are bf16 (1 cycle/row at 2.4 GHz; f32r is the same rate but
pays a slower 4-byte weight load). S matmuls are emitted one m-pair ahead of
the PV matmuls so the PE never waits on the ScalarE exp. S tiles are paired
[128, 2, 512] across two PSUM banks so one ACT instruction exps 1024 elems.
"""

import sys

if "/opt/trn_rl_repo" not in sys.path:
    sys.path.insert(0, "/opt/trn_rl_repo")

import ml_dtypes
import numpy as np

B = 4
C = 256
HID = 64
N = 4096          # H*W
P = 128           # SBUF partitions
NT = 512          # n-tile (matmul moving free dim)
N_NT = N // NT    # 8
MB = 128          # m-block (PV contraction tile)
N_MB = N // MB    # 32
NPAIR = N_MB // 2  # 16 m-pairs per n-tile
CA = C // P       # 2 c-chunks / c-blocks

_CACHE = {}


def _build_program():
    import concourse.bass as bass
    import concourse.mybir as mybir
    from concourse import tile

    f32 = mybir.dt.float32
    f32r = mybir.dt.float32r
    bf16 = mybir.dt.bfloat16
    AF = mybir.ActivationFunctionType

    nc = bass.Bass("TRN2", target_bir_lowering=False, debug=False)

    xq_d = nc.dram_tensor("xq", (C, N), bf16, kind="ExternalInput")
    xkv_d = nc.dram_tensor("xkv", (C, N), bf16, kind="ExternalInput")
    wqT_d = nc.dram_tensor("wqT", (C, HID), bf16, kind="ExternalInput")
    wkT_d = nc.dram_tensor("wkT", (C, HID), bf16, kind="ExternalInput")
    wvT_d = nc.dram_tensor("wvT", (C, C), bf16, kind="ExternalInput")
    bq_d = nc.dram_tensor("bq", (P, 1), f32, kind="ExternalInput")   # bq tiled x2
    bk_d = nc.dram_tensor("bk", (P, 1), f32, kind="ExternalInput")   # bk tiled x2
    gbv_d = nc.dram_tensor("gbv", (C, 1), f32, kind="ExternalInput")      # gamma * bv
    rgam_d = nc.dram_tensor("rgam", (1, 1), f32, kind="ExternalInput")    # 1 / gamma
    x2tb_d = nc.dram_tensor("x2tb", (N, C), bf16, kind="ExternalInput")   # bf16 xkv^T
    out_d = nc.dram_tensor("out", (C, N), f32, kind="ExternalOutput")

    # c = a*128 + p views
    xq_r = xq_d[:].rearrange("(a p) n -> p a n", p=P)
    xkv_r = xkv_d[:].rearrange("(a p) n -> p a n", p=P)
    wqT_r = wqT_d[:].rearrange("(a p) h -> p a h", p=P)
    wkT_r = wkT_d[:].rearrange("(a p) h -> p a h", p=P)
    wvT_r = wvT_d[:].rearrange("(a p) c -> p a c", p=P)
    gbv_r = gbv_d[:].rearrange("(a p) one -> p (a one)", p=P)
    out_r = out_d[:].rearrange("(a p) n -> p a n", p=P)
    x2t_r = x2tb_d[:].rearrange("(mb p) c -> p mb c", p=P)

    with tile.TileContext(nc) as tc:
        with (
            tc.tile_pool(name="const", bufs=1) as const,
            tc.tile_pool(name="xin", bufs=1) as xin,
            tc.tile_pool(name="qk", bufs=1) as qk,
            tc.tile_pool(name="vtp", bufs=1) as vtp,
            tc.tile_pool(name="work", bufs=3) as work,
            tc.tile_pool(name="ep", bufs=2) as ep,
            tc.tile_pool(name="dram", bufs=2, space="DRAM") as dram,
            tc.tile_pool(name="psum", bufs=1, space="PSUM") as psum,
        ):
            # ---- constants / weights (ACT HWDGE queue; wk first for k-proj) --
            wk_sb = const.tile([P, CA, HID], bf16, tag="wk")
            nc.scalar.dma_start(wk_sb[:], wkT_r)
            wq_sb = const.tile([P, CA, HID], bf16, tag="wq")
            nc.scalar.dma_start(wq_sb[:], wqT_r)
            wv_sb = const.tile([P, CA, C], bf16, tag="wv")
            nc.scalar.dma_start(wv_sb[:], wvT_r)
            bq_sb = const.tile([P, 1], f32, tag="bq")
            nc.scalar.dma_start(bq_sb[:], bq_d[:])
            bk_sb = const.tile([P, 1], f32, tag="bk")
            nc.scalar.dma_start(bk_sb[:], bk_d[:])
            gbv_sb = const.tile([P, CA], f32, tag="gbv")
            nc.scalar.dma_start(gbv_sb[:], gbv_r)
            rgam_sb = const.tile([1, 1], f32, tag="rgam")
            nc.scalar.dma_start(rgam_sb[:], rgam_d[:])
            ones_sb = const.tile([P, 1], bf16, tag="ones")
            nc.vector.memset(ones_sb[:], 1.0)
            onesb_sb = const.tile([1, P], bf16, tag="onesb")
            nc.vector.memset(onesb_sb[:], 1.0)
            # exp bias compensating the 2^-32 Ln range shift
            l2b_sb = const.tile([1, 1], f32, tag="l2b")
            nc.vector.memset(l2b_sb[:], -32.0 * float(np.log(2.0)))

            # ---- x loads (chunked for DMA/compute overlap) ----
            # xq chunk 0 first: it gates the very first q-projection.
            xq_sb = xin.tile([P, CA, N], bf16, tag="xq")
            xkv_sb = xin.tile([P, CA, N], bf16, tag="xkv")
            NCH = 1024
            for a in range(CA):
                nc.sync.dma_start(xq_sb[:, a, 0:NCH], xq_r[:, a, 0:NCH])
            for h in range(N // NCH):
                sl = slice(h * NCH, (h + 1) * NCH)
                for a in range(CA):
                    nc.sync.dma_start(xkv_sb[:, a, sl], xkv_r[:, a, sl])
            # X2^T tiles [m, c_in] (bf16) for the Y = X2 @ E matmuls
            # (transpose+cast on the host); SWDGE queue, parallel with sync.
            x2t_sb = vtp.tile([P, N_MB, C], bf16, tag="x2t")
            for mb in range(0, N_MB, 4):
                nc.gpsimd.dma_start(
                    x2t_sb[:, mb : mb + 4, :], x2t_r[:, mb : mb + 4, :]
                )
            for h in range(1, N // NCH):
                sl = slice(h * NCH, (h + 1) * NCH)
                for a in range(CA):
                    nc.sync.dma_start(xq_sb[:, a, sl], xq_r[:, a, sl])

            q_sb = qk.tile([HID, N], bf16, tag="q")
            k_sb = qk.tile([HID, N], bf16, tag="k")

            def _kproj(j):
                # k-projection for n-tiles (2j, 2j+1), packed in one PSUM bank:
                # rows 0:64 <- nt 2j, rows 64:128 <- nt 2j+1
                kp = psum.tile([P, 2, NT], f32, tag="stp", bufs=2, name=f"kp_{j}")
                for half, ntj in ((0, 2 * j), (1, 2 * j + 1)):
                    ntsl = slice(ntj * NT, (ntj + 1) * NT)
                    rows = slice(half * HID, half * HID + HID)
                    for a in range(CA):
                        nc.tensor.matmul(
                            kp[rows, 0, :],
                            lhsT=wk_sb[:, a, :],
                            rhs=xkv_sb[:, a, ntsl],
                            start=(a == 0),
                            stop=(a == CA - 1),
                        )
                    nc.vector.tensor_scalar_add(
                        k_sb[:, ntsl], kp[rows, 0, :], bk_sb[rows]
                    )

            def _qproj(i):
                # q-projection for n-tiles (2i, 2i+1) packed into one bank
                qp = psum.tile([P, NT], f32, tag="mp", bufs=1, name=f"qp_{i}")
                for half, nti in ((0, 2 * i), (1, 2 * i + 1)):
                    ntsl = slice(nti * NT, (nti + 1) * NT)
                    rows = slice(half * HID, half * HID + HID)
                    for a in range(CA):
                        nc.tensor.matmul(
                            qp[rows, :],
                            lhsT=wq_sb[:, a, :],
                            rhs=xq_sb[:, a, ntsl],
                            start=(a == 0),
                            stop=(a == CA - 1),
                        )
                    nc.vector.tensor_scalar_add(
                        q_sb[:, ntsl], qp[rows, :], bq_sb[rows]
                    )

            _kproj(0)
            _qproj(0)

            # ---- attention ----
            GP = 4            # m-pairs per denominator group (8 m-blocks)
            N_G = NPAIR // GP  # 4 groups per n-tile

            def _epi_a(nt, y0, y1, dp):
                # After nt's loop: free the y banks (DVE copies), and move
                # d/gamma off PSUM (ACT copy; f32r so the broadcast matmul can
                # consume it at full rate).
                yb0 = ep.tile([P, NT], bf16, tag="yb0", name=f"yb0_{nt}")
                nc.vector.tensor_scalar_add(yb0[:], y0[:], 0.0)
                yb1 = ep.tile([P, NT], bf16, tag="yb1", name=f"yb1_{nt}")
                nc.vector.tensor_scalar_add(yb1[:], y1[:], 0.0)
                # gamma/d = exp(-ln(d/gamma)) on ACT (ln/exp/copy share one
                # activation table set; avoids the 3.4us DVE reciprocal).
                # ACT Ln only covers |x| <= 2^64 while d/gamma reaches ~1e23,
                # so the host sends rgam = 2^-32/gamma and the Exp bias adds
                # -32*ln2 to compensate exactly.
                rd = ep.tile([1, NT], f32, tag="rd", name=f"rd_{nt}")
                nc.scalar.activation(rd[:], dp[:], AF.Ln, scale=rgam_sb[:])
                rb = ep.tile([1, NT], bf16, tag="rb", name=f"rb_{nt}")
                nc.scalar.activation(
                    rb[:], rd[:], AF.Exp, scale=-1.0, bias=l2b_sb[:]
                )
                return yb0, yb1, rb

            def _epi_bcast(nt, state):
                # Partition-broadcast gamma/d (1KB bf16) via a DRAM roundtrip.
                yb0, yb1, rb = state
                dscr = dram.tile([1, NT], bf16, tag="dscr", name=f"dscr_{nt}")
                nc.sync.dma_start(dscr[:], rb[:])
                rdb = ep.tile([P, NT], bf16, tag="rdb", name=f"rdb_{nt}")
                nc.sync.dma_start(rdb[:], dscr[:].broadcast_to((P, NT)))
                return yb0, yb1, rdb

            def _epi_b(nt, cb, state):
                # U[cb] = Wv[cb] @ Y  (2 accumulating matmuls), then
                # out[c, n] = xq + rdb[n] * U[c, n] + gamma*bv[c]
                yb0, yb1, rdb = state
                ntsl = slice(nt * NT, (nt + 1) * NT)
                ups = psum.tile([P, NT], f32, tag="mp", bufs=1, name=f"ups_{nt}_{cb}")
                nc.tensor.matmul(
                    ups[:], lhsT=wv_sb[:, 0, cb * P : (cb + 1) * P],
                    rhs=yb0[:], start=True, stop=False,
                )
                nc.tensor.matmul(
                    ups[:], lhsT=wv_sb[:, 1, cb * P : (cb + 1) * P],
                    rhs=yb1[:], start=False, stop=True,
                )
                t = ep.tile([P, NT], f32, tag="t", name=f"t_{nt}_{cb}")
                nc.vector.tensor_mul(t[:], ups[:], rdb[:])
                o = ep.tile([P, NT], f32, tag="o", name=f"o_{nt}_{cb}")
                nc.vector.scalar_tensor_tensor(
                    o[:],
                    in0=t[:],
                    scalar=gbv_sb[:, cb : cb + 1],
                    in1=xq_sb[:, cb, ntsl],
                    op0=mybir.AluOpType.add,
                    op1=mybir.AluOpType.add,
                )
                nc.sync.dma_start(out_r[:, cb, ntsl], o[:])

            prev = [None]   # (nt, state) of the previous n-tile

            for nt in range(N_NT):
                ntsl = slice(nt * NT, (nt + 1) * NT)
                y0 = psum.tile([P, NT], f32, tag="y", bufs=2, name=f"y0_{nt}")
                y1 = psum.tile([P, NT], f32, tag="y", bufs=2, name=f"y1_{nt}")
                ddt = psum.tile([P, NT], f32, tag="dd", bufs=1, name=f"dp_{nt}")
                dp = ddt[0:1, :]

                stp = {}

                def _smm(p, nt=nt, ntsl=ntsl, stp=stp):
                    # S^T pair p: two matmuls [m=128, n=512] into one 2-bank
                    # PSUM tile; contraction over HID=64 partitions.
                    s = psum.tile([P, 2, NT], f32, tag="stp", bufs=2,
                                  name=f"stp_{nt}_{p}")
                    for j in range(2):
                        mb = 2 * p + j
                        msl = slice(mb * MB, (mb + 1) * MB)
                        nc.tensor.matmul(
                            s[:, j, :],
                            lhsT=k_sb[:, msl],
                            rhs=q_sb[:, ntsl],
                            start=True,
                            stop=True,
                        )
                    stp[p] = s

                _smm(0)
                _smm(1)

                acc = None
                n_dmm = 0
                for p in range(NPAIR):
                    # exp of pair p (one ACT op over both PSUM banks)
                    ex = work.tile([P, 2, NT], bf16, tag="expst",
                                   name=f"ex_{nt}_{p}")
                    nc.scalar.activation(ex[:], stp.pop(p)[:], AF.Exp)

                    # hooks: k-proj (nt 0), q-proj, deferred epilogue of nt-1
                    if nt == 0 and p in (0, 4, 8) and (p // 4 + 1) < 4:
                        _kproj(p // 4 + 1)
                    if p == 1 and prev[0] is not None:
                        prev[0] = (prev[0][0], _epi_bcast(prev[0][0], prev[0][1]))
                    if nt % 2 == 1 and p == 1 and nt < N_NT - 1:
                        _qproj((nt + 1) // 2)
                    if p == 4 and prev[0] is not None:
                        _epi_b(prev[0][0], 0, prev[0][1])
                    if p == 8 and prev[0] is not None:
                        _epi_b(prev[0][0], 1, prev[0][1])
                        prev[0] = None

                    # next S pair (keeps the PE fed while ACT exps pair p)
                    if p + 2 < NPAIR:
                        _smm(p + 2)

                    # PV: y[c] += x2t[mb]^T @ exp(S^T[mb]) for both halves
                    for j in range(2):
                        mb = 2 * p + j
                        first, last = (mb == 0), (mb == N_MB - 1)
                        nc.tensor.matmul(
                            y0[:], lhsT=x2t_sb[:, mb, 0:P], rhs=ex[:, j, :],
                            start=first, stop=last,
                        )
                        nc.tensor.matmul(
                            y1[:], lhsT=x2t_sb[:, mb, P:C], rhs=ex[:, j, :],
                            start=first, stop=last,
                        )

                    # denominator: bf16 running sum over the group's pairs,
                    # then 2 ones-matmuls per group accumulated into dp
                    if p % GP == 0:
                        acc = ex
                    else:
                        s_ = work.tile([P, 2, NT], bf16, tag=f"dacc{p % 2}",
                                       bufs=2, name=f"ds_{nt}_{p}")
                        nc.vector.tensor_add(s_[:], acc[:], ex[:])
                        acc = s_
                    if (p + 1) % GP == 0:
                        for j in range(2):
                            n_dmm += 1
                            nc.tensor.matmul(
                                dp[:], lhsT=ones_sb[:], rhs=acc[:, j, :],
                                start=(n_dmm == 1), stop=(n_dmm == 2 * N_G),
                            )
                        acc = None

                state = _epi_a(nt, y0, y1, dp)
                prev[0] = (nt, state)

            state = _epi_bcast(prev[0][0], prev[0][1])
            _epi_b(prev[0][0], 0, state)
            _epi_b(prev[0][0], 1, state)

    return nc


def _split_excess_waits(nc):
    """The pinned walrus build only encodes 1 sync-wait per instruction;
    newer concourse attaches more. Hoist excess waits onto same-engine NoOps
    inserted immediately before the over-limit instruction (semantically
    identical: same engine, same program position)."""
    import concourse.mybir as mybir
    import bass_rust

    ctr = 0
    for bbl in nc.m.functions[0].blocks:
        il = bbl.instructions
        i = 0
        while i < len(il):
            inst = il[i]
            si = inst.sync_info
            limit = 1
            if si is not None and len(si.on_wait) > limit:
                waits = list(si.on_wait)
                extra = waits[limit:]
                for j in range(0, len(extra), 1):
                    nop = mybir.InstNoOp(name=f"I-wsplit-{ctr}", ins=[], outs=[])
                    ctr += 1
                    nop.engine = inst.engine
                    nop.sync_info = bass_rust.SyncInfo(
                        on_wait=[extra[j]], on_update=[]
                    )
                    il.insert(i, nop)
                    i += 1
                si.on_wait = waits[:limit]
                inst.sync_info = si
            i += 1
    return ctr


def _get_program():
    if "nc" not in _CACHE:
        _CACHE["nc"] = _build_program()
    return _CACHE["nc"]


def _get_program_hw():
    """Program with the walrus sync-wait workaround applied (breaks CoreSim's
    race detector, so only applied for hardware runs)."""
    nc = _get_program()
    if not _CACHE.get("split_done"):
        _split_excess_waits(nc)
        _CACHE["split_done"] = True
    return nc


def _make_in_maps(x1, x2, Wq, bq, Wk, bk, Wv, bv, gamma):
    g = float(np.asarray(gamma).reshape(-1)[0])
    bf = ml_dtypes.bfloat16
    shared = {
        "wqT": np.ascontiguousarray(Wq.T).astype(bf),
        "wkT": np.ascontiguousarray(Wk.T).astype(bf),
        "wvT": np.ascontiguousarray(Wv.T).astype(bf),
        "bq": np.tile(np.asarray(bq, dtype=np.float32).reshape(HID, 1), (2, 1)),
        "bk": np.tile(np.asarray(bk, dtype=np.float32).reshape(HID, 1), (2, 1)),
        "gbv": (g * np.asarray(bv, dtype=np.float32)).reshape(C, 1),
        "rgam": np.array(
            [[2.0**-32 / g if g != 0.0 else 0.0]], dtype=np.float32
        ),
    }
    in_maps = []
    for d in range(2):
        src_q, src_kv = (x1, x2) if d == 0 else (x2, x1)
        for b in range(B):
            xkv_f32 = np.ascontiguousarray(src_kv[b].reshape(C, N), dtype=np.float32)
            in_maps.append(
                {
                    "xq": np.ascontiguousarray(
                        src_q[b].reshape(C, N), dtype=np.float32
                    ).astype(bf),
                    "xkv": xkv_f32.astype(bf),
                    "x2tb": np.ascontiguousarray(xkv_f32.T).astype(bf),
                    **shared,
                }
            )
    return in_maps


def kernel(x1, x2, Wq, bq, Wk, bk, Wv, bv, gamma, _want_results=False):
    x1 = np.asarray(x1, dtype=np.float32)
    x2 = np.asarray(x2, dtype=np.float32)
    nc = _get_program_hw()
    in_maps = _make_in_maps(x1, x2, Wq, bq, Wk, bk, Wv, bv, gamma)

    from concourse.bass_utils import run_bass_kernel_spmd

    res = run_bass_kernel_spmd(nc, in_maps, core_ids=list(range(2 * B)))
    outs = [r["out"].reshape(C, 64, 64) for r in res.results]
    out1 = np.stack(outs[:B]).astype(np.float32)
    out2 = np.stack(outs[B:]).astype(np.float32)
    if _want_results:
        return (out1, out2), res
    return (out1, out2)


# revision 24
# speedup vs baseline: 1.0740x; 1.0347x over previous
"""Trainium2 Bass kernel for nn_CrossModalAttention.

Problem: bidirectional cross-attention between two (B, C, H, W) feature maps.
  B=4, C=256, H=W=64 -> N=4096 pixels, HID=64.
  For each direction:  q = Wq@xq, k = Wk@xkv, v = Wv@xkv (1x1 convs),
  attn = softmax_m(q^T k), out = xq + gamma * (v @ attn^T).

Sharding: 2 directions x 4 batches = 8 independent units, one per NeuronCore.

Per-core layout: compute S^T tiles [m(part)=128, n(free)=512] via
matmul(lhsT=k_tile, rhs=q_tile) (contraction over HID=64 on partitions), exp on
ScalarE (logits are bounded ~ +-56, so exp in f32 needs no max-subtraction),
then accumulate Y[c, n] = sum_m x2T[m, c]^T expS^T[m, n] in PSUM across the m
blocks; U = Wv @ Y afterwards (v-projection folded past the attention sum by
associativity). Final: out = xq + (gamma/d)*U + gamma*bv.

All matmul operands are bf16 (1 cycle/row at 2.4 GHz; f32r is the same rate but
pays a slower 4-byte weight load). S matmuls are emitted one m-pair ahead of
the PV matmuls so the PE never waits on the ScalarE exp. S tiles are paired
[128, 2, 512] across two PSUM banks so one ACT instruction exps 1024 elems.
"""

import sys

if "/opt/trn_rl_repo" not in sys.path:
    sys.path.insert(0, "/opt/trn_rl_repo")

import ml_dtypes
import numpy as np

B = 4
C = 256
HID = 64
N = 4096          # H*W
P = 128           # SBUF partitions
NT = 512          # n-tile (matmul moving free dim)
N_NT = N // NT    # 8
MB = 128          # m-block (PV contraction tile)
N_MB = N // MB    # 32
NPAIR = N_MB // 2  # 16 m-pairs per n-tile
CA = C // P       # 2 c-chunks / c-blocks

_CACHE = {}


def _build_program():
    import concourse.bass as bass
    import concourse.mybir as mybir
    from concourse import tile

    f32 = mybir.dt.float32
    f32r = mybir.dt.float32r
    bf16 = mybir.dt.bfloat16
    AF = mybir.ActivationFunctionType

    nc = bass.Bass("TRN2", target_bir_lowering=False, debug=False)

    xq_d = nc.dram_tensor("xq", (C, N), bf16, kind="ExternalInput")
    xkv_d = nc.dram_tensor("xkv", (C, N), bf16, kind="ExternalInput")
    wqT_d = nc.dram_tensor("wqT", (C, HID), bf16, kind="ExternalInput")
    wkT_d = nc.dram_tensor("wkT", (C, HID), bf16, kind="ExternalInput")
    wvT_d = nc.dram_tensor("wvT", (C, C), bf16, kind="ExternalInput")
    bq_d = nc.dram_tensor("bq", (P, 1), f32, kind="ExternalInput")   # bq tiled x2
    bk_d = nc.dram_tensor("bk", (P, 1), f32, kind="ExternalInput")   # bk tiled x2
    gbv_d = nc.dram_tensor("gbv", (C, 1), f32, kind="ExternalInput")      # gamma * bv
    rgam_d = nc.dram_tensor("rgam", (1, 1), f32, kind="ExternalInput")    # 1 / gamma
    x2tb_d = nc.dram_tensor("x2tb", (N, C), bf16, kind="ExternalInput")   # bf16 xkv^T
    out_d = nc.dram_tensor("out", (C, N), f32, kind="ExternalOutput")

    # c = a*128 + p views
    xq_r = xq_d[:].rearrange("(a p) n -> p a n", p=P)
    xkv_r = xkv_d[:].rearrange("(a p) n -> p a n", p=P)
    wqT_r = wqT_d[:].rearrange("(a p) h -> p a h", p=P)
    wkT_r = wkT_d[:].rearrange("(a p) h -> p a h", p=P)
    wvT_r = wvT_d[:].rearrange("(a p) c -> p a c", p=P)
    gbv_r = gbv_d[:].rearrange("(a p) one -> p (a one)", p=P)
    out_r = out_d[:].rearrange("(a p) n -> p a n", p=P)
    x2t_r = x2tb_d[:].rearrange("(mb p) c -> p mb c", p=P)

    with tile.TileContext(nc) as tc:
        with (
            tc.tile_pool(name="const", bufs=1) as const,
            tc.tile_pool(name="xin", bufs=1) as xin,
            tc.tile_pool(name="qk", bufs=1) as qk,
            tc.tile_pool(name="vtp", bufs=1) as vtp,
            tc.tile_pool(name="work", bufs=3) as work,
            tc.tile_pool(name="ep", bufs=2) as ep,
            tc.tile_pool(name="dram", bufs=2, space="DRAM") as dram,
            tc.tile_pool(name="psum", bufs=1, space="PSUM") as psum,
        ):
            # ---- constants / weights (ACT HWDGE queue; wk first for k-proj) --
            wk_sb = const.tile([P, CA, HID], bf16, tag="wk")
            nc.scalar.dma_start(wk_sb[:], wkT_r)
            wq_sb = const.tile([P, CA, HID], bf16, tag="wq")
            nc.scalar.dma_start(wq_sb[:], wqT_r)
            wv_sb = const.tile([P, CA, C], bf16, tag="wv")
            nc.scalar.dma_start(wv_sb[:], wvT_r)
            bq_sb = const.tile([P, 1], f32, tag="bq")
            nc.scalar.dma_start(bq_sb[:], bq_d[:])
            bk_sb = const.tile([P, 1], f32, tag="bk")
            nc.scalar.dma_start(bk_sb[:], bk_d[:])
            gbv_sb = const.tile([P, CA], f32, tag="gbv")
            nc.scalar.dma_start(gbv_sb[:], gbv_r)
            rgam_sb = const.tile([1, 1], f32, tag="rgam")
            nc.scalar.dma_start(rgam_sb[:], rgam_d[:])
            ones_sb = const.tile([P, 1], bf16, tag="ones")
            nc.vector.memset(ones_sb[:], 1.0)
            onesb_sb = const.tile([1, P], bf16, tag="onesb")
            nc.vector.memset(onesb_sb[:], 1.0)
            # exp bias compensating the 2^-32 Ln range shift
            l2b_sb = const.tile([1, 1], f32, tag="l2b")
            nc.vector.memset(l2b_sb[:], -32.0 * float(np.log(2.0)))

            # ---- x loads (chunked for DMA/compute overlap) ----
            # xq chunk 0 first: it gates the very first q-projection.
            xq_sb = xin.tile([P, CA, N], bf16, tag="xq")
            xkv_sb = xin.tile([P, CA, N], bf16, tag="xkv")
            # Input DMA is spread over two queues so transfers overlap:
            #   sync:   xq c0, xkv c0, xq c1-3
            #   gpsimd: xkv c1, x2t chunk0, xkv c2-3, x2t chunks 1-7
            # Each chunk lands just before its first consumer (kp(j) needs
            # xkv cj; PV(0) needs x2t chunk0).
            NCH = 1024
            x2t_sb = vtp.tile([P, N_MB, C], bf16, tag="x2t")

            def _xkv_load(h, eng):
                sl = slice(h * NCH, (h + 1) * NCH)
                for a in range(CA):
                    eng.dma_start(xkv_sb[:, a, sl], xkv_r[:, a, sl])

            def _x2t_load(ch):
                mb = 4 * ch
                nc.gpsimd.dma_start(
                    x2t_sb[:, mb : mb + 4, :], x2t_r[:, mb : mb + 4, :]
                )

            for a in range(CA):
                nc.sync.dma_start(xq_sb[:, a, 0:NCH], xq_r[:, a, 0:NCH])
            _xkv_load(0, nc.sync)
            _xkv_load(1, nc.gpsimd)
            _x2t_load(0)
            _xkv_load(2, nc.gpsimd)
            _xkv_load(3, nc.gpsimd)
            for ch in range(1, 8):
                _x2t_load(ch)
            for h in range(1, N // NCH):
                sl = slice(h * NCH, (h + 1) * NCH)
                for a in range(CA):
                    nc.sync.dma_start(xq_sb[:, a, sl], xq_r[:, a, sl])

            q_sb = qk.tile([HID, N], bf16, tag="q")
            k_sb = qk.tile([HID, N], bf16, tag="k")

            def _kproj(j):
                # k-projection for n-tiles (2j, 2j+1), packed in one PSUM bank:
                # rows 0:64 <- nt 2j, rows 64:128 <- nt 2j+1
                kp = psum.tile([P, 2, NT], f32, tag="stp", bufs=2, name=f"kp_{j}")
                for half, ntj in ((0, 2 * j), (1, 2 * j + 1)):
                    ntsl = slice(ntj * NT, (ntj + 1) * NT)
                    rows = slice(half * HID, half * HID + HID)
                    for a in range(CA):
                        nc.tensor.matmul(
                            kp[rows, 0, :],
                            lhsT=wk_sb[:, a, :],
                            rhs=xkv_sb[:, a, ntsl],
                            start=(a == 0),
                            stop=(a == CA - 1),
                        )
                    nc.vector.tensor_scalar_add(
                        k_sb[:, ntsl], kp[rows, 0, :], bk_sb[rows]
                    )

            def _qproj(i):
                # q-projection for n-tiles (2i, 2i+1) packed into one bank
                qp = psum.tile([P, NT], f32, tag="mp", bufs=1, name=f"qp_{i}")
                for half, nti in ((0, 2 * i), (1, 2 * i + 1)):
                    ntsl = slice(nti * NT, (nti + 1) * NT)
                    rows = slice(half * HID, half * HID + HID)
                    for a in range(CA):
                        nc.tensor.matmul(
                            qp[rows, :],
                            lhsT=wq_sb[:, a, :],
                            rhs=xq_sb[:, a, ntsl],
                            start=(a == 0),
                            stop=(a == CA - 1),
                        )
                    nc.vector.tensor_scalar_add(
                        q_sb[:, ntsl], qp[rows, :], bq_sb[rows]
                    )

            _kproj(0)
            _qproj(0)

            # ---- attention ----
            GP = 16           # m-pairs per denominator group (one group/nt)
            N_G = NPAIR // GP  # 1

            def _epi_a(nt, y0, y1, dp, last=False):
                # After nt's loop: free the y banks, and start the d ->
                # gamma/d chain. The y copies go on DVE mid-loop (ACT would
                # delay the next n-tile's exps); for the last n-tile they go
                # on the now-idle ACT to shorten the tail chain.
                yb0 = ep.tile([P, NT], bf16, tag="yb0", name=f"yb0_{nt}")
                yb1 = ep.tile([P, NT], bf16, tag="yb1", name=f"yb1_{nt}")
                if last:
                    nc.scalar.copy(yb0[:], y0[:])
                    nc.scalar.copy(yb1[:], y1[:])
                else:
                    nc.vector.tensor_scalar_add(yb0[:], y0[:], 0.0)
                    nc.vector.tensor_scalar_add(yb1[:], y1[:], 0.0)
                # gamma/d = exp(-ln(d/gamma)) on ACT (ln/exp/copy share one
                # activation table set; avoids the 3.4us DVE reciprocal).
                # ACT Ln only covers |x| <= 2^64 while d/gamma reaches ~1e23,
                # so the host sends rgam = 2^-32/gamma and the Exp bias adds
                # -32*ln2 to compensate exactly.
                rd = ep.tile([1, NT], f32, tag="rd", name=f"rd_{nt}")
                nc.scalar.activation(rd[:], dp[:], AF.Ln, scale=rgam_sb[:])
                rb = ep.tile([1, NT], bf16, tag="rb", name=f"rb_{nt}")
                nc.scalar.activation(
                    rb[:], rd[:], AF.Exp, scale=-1.0, bias=l2b_sb[:]
                )
                return yb0, yb1, rb

            def _epi_bcast(nt, state):
                # Partition-broadcast gamma/d (1KB bf16) via a DRAM roundtrip.
                yb0, yb1, rb = state
                dscr = dram.tile([1, NT], bf16, tag="dscr", name=f"dscr_{nt}")
                nc.sync.dma_start(dscr[:], rb[:])
                rdb = ep.tile([P, NT], bf16, tag="rdb", name=f"rdb_{nt}")
                nc.sync.dma_start(rdb[:], dscr[:].broadcast_to((P, NT)))
                return yb0, yb1, rdb

            def _epi_b(nt, cb, state):
                # U[cb] = Wv[cb] @ Y  (2 accumulating matmuls), then
                # out[c, n] = xq + rdb[n] * U[c, n] + gamma*bv[c]
                yb0, yb1, rdb = state
                ntsl = slice(nt * NT, (nt + 1) * NT)
                ups = psum.tile([P, NT], f32, tag="mp", bufs=1, name=f"ups_{nt}_{cb}")
                nc.tensor.matmul(
                    ups[:], lhsT=wv_sb[:, 0, cb * P : (cb + 1) * P],
                    rhs=yb0[:], start=True, stop=False,
                )
                nc.tensor.matmul(
                    ups[:], lhsT=wv_sb[:, 1, cb * P : (cb + 1) * P],
                    rhs=yb1[:], start=False, stop=True,
                )
                t = ep.tile([P, NT], f32, tag="t", name=f"t_{nt}_{cb}")
                nc.vector.tensor_mul(t[:], ups[:], rdb[:])
                o = ep.tile([P, NT], f32, tag="o", name=f"o_{nt}_{cb}")
                nc.vector.scalar_tensor_tensor(
                    o[:],
                    in0=t[:],
                    scalar=gbv_sb[:, cb : cb + 1],
                    in1=xq_sb[:, cb, ntsl],
                    op0=mybir.AluOpType.add,
                    op1=mybir.AluOpType.add,
                )
                nc.sync.dma_start(out_r[:, cb, ntsl], o[:])

            prev = [None]   # (nt, state) of the previous n-tile

            for nt in range(N_NT):
                ntsl = slice(nt * NT, (nt + 1) * NT)
                y0 = psum.tile([P, NT], f32, tag="y", bufs=2, name=f"y0_{nt}")
                y1 = psum.tile([P, NT], f32, tag="y", bufs=2, name=f"y1_{nt}")
                ddt = psum.tile([P, NT], f32, tag="dd", bufs=1, name=f"dp_{nt}")
                dp = ddt[0:1, :]

                stp = {}

                def _smm(p, nt=nt, ntsl=ntsl, stp=stp):
                    # S^T pair p: two matmuls [m=128, n=512] into one 2-bank
                    # PSUM tile; contraction over HID=64 partitions.
                    s = psum.tile([P, 2, NT], f32, tag="stp", bufs=2,
                                  name=f"stp_{nt}_{p}")
                    for j in range(2):
                        mb = 2 * p + j
                        msl = slice(mb * MB, (mb + 1) * MB)
                        nc.tensor.matmul(
                            s[:, j, :],
                            lhsT=k_sb[:, msl],
                            rhs=q_sb[:, ntsl],
                            start=True,
                            stop=True,
                        )
                    stp[p] = s

                _smm(0)
                _smm(1)

                acc = None
                n_dmm = 0
                for p in range(NPAIR):
                    # exp of pair p (one ACT op over both PSUM banks)
                    ex = work.tile([P, 2, NT], bf16, tag="expst",
                                   name=f"ex_{nt}_{p}")
                    nc.scalar.activation(ex[:], stp.pop(p)[:], AF.Exp)

                    # hooks: k-proj (nt 0), q-proj, deferred epilogue of nt-1
                    if nt == 0 and p in (0, 4, 8) and (p // 4 + 1) < 4:
                        _kproj(p // 4 + 1)
                    if p == 1 and prev[0] is not None:
                        prev[0] = (prev[0][0], _epi_bcast(prev[0][0], prev[0][1]))
                    if nt % 2 == 1 and p == 1 and nt < N_NT - 1:
                        _qproj((nt + 1) // 2)
                    if p == 4 and prev[0] is not None:
                        _epi_b(prev[0][0], 0, prev[0][1])
                    if p == 8 and prev[0] is not None:
                        _epi_b(prev[0][0], 1, prev[0][1])
                        prev[0] = None

                    # next S pair (keeps the PE fed while ACT exps pair p)
                    if p + 2 < NPAIR:
                        _smm(p + 2)

                    # PV: y[c] += x2t[mb]^T @ exp(S^T[mb]) for both halves
                    for j in range(2):
                        mb = 2 * p + j
                        first, last = (mb == 0), (mb == N_MB - 1)
                        nc.tensor.matmul(
                            y0[:], lhsT=x2t_sb[:, mb, 0:P], rhs=ex[:, j, :],
                            start=first, stop=last,
                        )
                        nc.tensor.matmul(
                            y1[:], lhsT=x2t_sb[:, mb, P:C], rhs=ex[:, j, :],
                            start=first, stop=last,
                        )

                    # denominator: bf16 running sum over the group's pairs,
                    # then 2 ones-matmuls per group accumulated into dp
                    if p % GP == 0:
                        acc = ex
                    else:
                        s_ = work.tile([P, 2, NT], bf16, tag=f"dacc{p % 2}",
                                       bufs=2, name=f"ds_{nt}_{p}")
                        nc.vector.tensor_add(s_[:], acc[:], ex[:])
                        acc = s_
                    if (p + 1) % GP == 0:
                        for j in range(2):
                            n_dmm += 1
                            nc.tensor.matmul(
                                dp[:], lhsT=ones_sb[:], rhs=acc[:, j, :],
                                start=(n_dmm == 1), stop=(n_dmm == 2 * N_G),
                            )
                        acc = None

                state = _epi_a(nt, y0, y1, dp, last=(nt == N_NT - 1))
                prev[0] = (nt, state)

            state = _epi_bcast(prev[0][0], prev[0][1])
            _epi_b(prev[0][0], 0, state)
            _epi_b(prev[0][0], 1, state)

    return nc


def _split_excess_waits(nc):
    """The pinned walrus build only encodes 1 sync-wait per instruction;
    newer concourse attaches more. Hoist excess waits onto same-engine NoOps
    inserted immediately before the over-limit instruction (semantically
    identical: same engine, same program position)."""
    import concourse.mybir as mybir
    import bass_rust

    ctr = 0
    for bbl in nc.m.functions[0].blocks:
        il = bbl.instructions
        i = 0
        while i < len(il):
            inst = il[i]
            si = inst.sync_info
            limit = 1
            if si is not None and len(si.on_wait) > limit:
                waits = list(si.on_wait)
                extra = waits[limit:]
                for j in range(0, len(extra), 1):
                    nop = mybir.InstNoOp(name=f"I-wsplit-{ctr}", ins=[], outs=[])
                    ctr += 1
                    nop.engine = inst.engine
                    nop.sync_info = bass_rust.SyncInfo(
                        on_wait=[extra[j]], on_update=[]
                    )
                    il.insert(i, nop)
                    i += 1
                si.on_wait = waits[:limit]
                inst.sync_info = si
            i += 1
    return ctr


def _get_program():
    if "nc" not in _CACHE:
        _CACHE["nc"] = _build_program()
    return _CACHE["nc"]


def _get_program_hw():
    """Program with the walrus sync-wait workaround applied (breaks CoreSim's
    race detector, so only applied for hardware runs)."""
    nc = _get_program()
    if not _CACHE.get("split_done"):
        _split_excess_waits(nc)
        _CACHE["split_done"] = True
    return nc


def _make_in_maps(x1, x2, Wq, bq, Wk, bk, Wv, bv, gamma):
    g = float(np.asarray(gamma).reshape(-1)[0])
    bf = ml_dtypes.bfloat16
    shared = {
        "wqT": np.ascontiguousarray(Wq.T).astype(bf),
        "wkT": np.ascontiguousarray(Wk.T).astype(bf),
        "wvT": np.ascontiguousarray(Wv.T).astype(bf),
        "bq": np.tile(np.asarray(bq, dtype=np.float32).reshape(HID, 1), (2, 1)),
        "bk": np.tile(np.asarray(bk, dtype=np.float32).reshape(HID, 1), (2, 1)),
        "gbv": (g * np.asarray(bv, dtype=np.float32)).reshape(C, 1),
        "rgam": np.array(
            [[2.0**-32 / g if g != 0.0 else 0.0]], dtype=np.float32
        ),
    }
    in_maps = []
    for d in range(2):
        src_q, src_kv = (x1, x2) if d == 0 else (x2, x1)
        for b in range(B):
            xkv_f32 = np.ascontiguousarray(src_kv[b].reshape(C, N), dtype=np.float32)
            in_maps.append(
                {
                    "xq": np.ascontiguousarray(
                        src_q[b].reshape(C, N), dtype=np.float32
                    ).astype(bf),
                    "xkv": xkv_f32.astype(bf),
                    "x2tb": np.ascontiguousarray(xkv_f32.T).astype(bf),
                    **shared,
                }
            )
    return in_maps


def kernel(x1, x2, Wq, bq, Wk, bk, Wv, bv, gamma, _want_results=False):
    x1 = np.asarray(x1, dtype=np.float32)
    x2 = np.asarray(x2, dtype=np.float32)
    nc = _get_program_hw()
    in_maps = _make_in_maps(x1, x2, Wq, bq, Wk, bk, Wv, bv, gamma)

    from concourse.bass_utils import run_bass_kernel_spmd

    res = run_bass_kernel_spmd(nc, in_maps, core_ids=list(range(2 * B)))
    outs = [r["out"].reshape(C, 64, 64) for r in res.results]
    out1 = np.stack(outs[:B]).astype(np.float32)
    out2 = np.stack(outs[B:]).astype(np.float32)
    if _want_results:
        return (out1, out2), res
    return (out1, out2)
